# revision 1
# baseline (speedup 1.0000x reference)
"""Trainium2 Bass kernel for nn_BertClassifier_77309411685 (V7).

Data-parallel over 8 NeuronCores: each core handles 256 samples; the small
base linear and 12 expert heads are replicated.

Strategy:
  * fp16 end-to-end (host-cast); PSUM accumulation stays fp32.
  * span gather: ONE indirect DMA per half-batch of 128 samples — 8
    contiguous rows from `start` as a single 12KB descriptor per sample
    (spans always fit inside the sample's S rows, so no bounds handling).
    Rows past the span are real data, killed by masks later.  Single-index
    offset APs only: the HW DGE misreads multi-index offset APs.
  * masked mean on the PE: per half, 8 accumulating matmuls with stationary
    diag(mask_j/len) (built on DVE from a shipped diag(1/len) and 0/1 span
    masks) against the gathered rows -> center in PSUM, already averaged.
  * the 2 static context rows are host-sliced and shipped pre-transposed in
    featT layout, DMA'd straight into the featT tile.
  * base linear: ctx k-chunks run as soon as their weights land (DMA'd
    before the gathers queue); center k-chunks per half close the PSUM
    accumulation; bias+relu fused in the PSUM->SBUF activation.
  * expert heads: all 12 experts at once with bias folded via a ones row;
    per-sample selection by is_equal mask + strided reduce.
"""

import numpy as np
from contextlib import ExitStack

import concourse.bass as bass
import concourse.tile as tile
from concourse import bacc, mybir
from concourse.bass import IndirectOffsetOnAxis
from concourse.bass_utils import run_bass_kernel_spmd

F32 = mybir.dt.float32
F16 = mybir.dt.float16
I32 = mybir.dt.int32

B, S, H = 2048, 256, 768
INNER, NB_CTX, NB_EXPERTS, NB_LABELS = 256, 2, 12, 3
NCORES = 8
BC = B // NCORES             # 256 samples per core
F3H = (NB_CTX + 1) * H       # 2304
KC = F3H // 128              # 18 contraction chunks
HC = H // 128                # 6 chunks per feature block
NE = NB_EXPERTS * NB_LABELS  # 36
SPAN = 8
HROWS = 128 * S              # rows per half-batch tensor

# The reference picks 2 static context positions host-side with this exact rng.
CTX_IDX = [int(v) for v in np.random.default_rng(seed=0).choice(np.arange(S), size=NB_CTX)]



def _build():
    nc = bacc.Bacc(
        "TRN2",
        target_bir_lowering=False,
        debug=False,
        enable_asserts=False,
        num_devices=NCORES,
    )
    embs = [nc.dram_tensor(f"emb{h}", [HROWS, H], F16, kind="ExternalInput").ap()
            for h in range(2)]
    gidx = nc.dram_tensor("gidx", [128, 2], I32, kind="ExternalInput").ap()
    wbT = nc.dram_tensor("wbT", [F3H, INNER], F16, kind="ExternalInput").ap()
    ctxT = nc.dram_tensor("ctxT", [128, NB_CTX * HC * 256], F16, kind="ExternalInput").ap()
    # c16: diag(1/len) h0/h1 [0:256) + identity [256:384) + wexpA [384:420)
    #      + wexpB [420:456)
    c16 = nc.dram_tensor("c16", [128, 3 * 128 + 2 * NE], F16, kind="ExternalInput").ap()
    # c32: io36 [0:36) + categories-as-float [36:38) + span masks [38:54)
    #      + b_base (t p) layout [54:56)
    c32 = nc.dram_tensor("c32", [128, NE + 2 + 16 + 2], F32, kind="ExternalInput").ap()
    # c1: ones row [0:256) + wexp bias row [256:292)
    c1 = nc.dram_tensor("c1", [1, 256 + NE], F16, kind="ExternalInput").ap()
    out = nc.dram_tensor("out", [BC, NB_LABELS], F32, kind="ExternalOutput").ap()

    with tile.TileContext(nc) as tc, ExitStack() as ctx:
        pool = ctx.enter_context(tc.tile_pool(name="main", bufs=1))
        pst = ctx.enter_context(tc.tile_pool(name="pst", bufs=1, space="PSUM"))
        gpool = pool
        spool = pool
        psh = pst
        ps36p = pst

        # --- phase 0: tiny front-of-queue loads the gathers depend on ---
        # gidx rides the gpsimd (SWDGE) queue: the Pool engine tracks its own
        # DMA completion, so the gather descriptor generation that reads it
        # does not wait on a cross-engine semaphore round-trip.
        gidx_t = pool.tile([128, 2], I32)
        nc.gpsimd.dma_start(gidx_t[:], gidx[:, :])

        # --- phase 1: span gathers — one op per half, 8 contiguous rows per
        # sample as a single 12KB descriptor (single-index offset AP only:
        # the HW DGE misreads multi-index offset APs) ---
        g_h = []
        for h in range(2):
            g = gpool.tile([128, SPAN * H], F16, tag=f"g{h}", bufs=1)
            nc.gpsimd.indirect_dma_start(
                out=g[:], out_offset=None, in_=embs[h],
                in_offset=IndirectOffsetOnAxis(ap=gidx_t[:, h:h + 1], axis=0),
            )
            g_h.append(g)

        # --- phase 2: big weight streams ride the ACT engine's separate
        # HWDGE queue so they start immediately; packed small consts follow
        # gidx on the sync queue ---
        featT = pool.tile([128, KC * 256], F16)
        wbT_t = pool.tile([128, KC * INNER], F16)
        wbT_c = wbT.rearrange("(p x) m -> p (x m)", p=128)
        # ctx-chunk weights first (phase 3b), then the static ctx rows
        # (pre-transposed in exact featT layout), center weights last
        nc.sync.dma_start(wbT_t[:, HC * INNER:], wbT_c[:, HC * INNER:])
        nc.sync.dma_start(featT[:, HC * 256:KC * 256], ctxT[:, :])
        nc.sync.dma_start(wbT_t[:, :HC * INNER], wbT_c[:, :HC * INNER])

        c16_t = pool.tile([128, 3 * 128 + 2 * NE], F16)
        nc.scalar.dma_start(c16_t[:], c16[:, :])
        dcst_t = c16_t  # diag/identity live in the packed blob
        wexpA = c16_t[:, 384:384 + NE]
        wexpB = c16_t[:, 384 + NE:384 + 2 * NE]
        c32_t = pool.tile([128, NE + 2 + 16 + 2], F32)
        nc.scalar.dma_start(c32_t[:], c32[:, :])
        cst32_t = c32_t
        io36f = c32_t[:, 0:NE]
        catf = c32_t[:, NE:NE + 2]
        bb_t = c32_t[:, NE + 18:NE + 20]  # bb_t[p, t] = b_base[t*128 + p]
        c1_t = pool.tile([1, 256 + NE], F16)
        nc.scalar.dma_start(c1_t[:], c1[:, :])
        ones1 = c1_t[:, 0:256]
        wexpC = c1_t[:, 256:256 + NE]

        # PE warm-up: the HAM clock gate releases after ~3.4us of sustained
        # activity; a burst of throwaway matmuls on the already-loaded const
        # tile warms the array before the real matmuls arrive.
        warm = pst.tile([128, 256], F32, tag="psb", bufs=1)
        for w in range(8):
            nc.tensor.matmul(warm[:], lhsT=c16_t[:, 0:128], rhs=c16_t[:, 0:256],
                             start=(w == 0), stop=(w == 7))

        # --- phase 3b/3c/4 interleaved per half ---
        # All base-linear matmuls are N=128, grouped per (m-tile, half): the
        # ctx chunks open each accumulation group (overlapping the gathers),
        # the center chunks close it.
        # one PSUM bank per (half, m-tile): all four base-linear accumulation
        # groups are open concurrently and a bank admits only one open group
        accs_h = [[psh.tile([128, 128], F32, tag=f"acc{h}{mt}", bufs=1,
                            name=f"acc{h}{mt}") for mt in range(2)]
                  for h in range(2)]

        def ctx_mms(h):
            for c in range(HC, KC):
                for mt in range(2):
                    nc.tensor.matmul(
                        accs_h[h][mt][:],
                        lhsT=wbT_t[:, c * INNER + mt * 128: c * INNER + (mt + 1) * 128],
                        rhs=featT[:, c * 256 + h * 128: c * 256 + h * 128 + 128],
                        start=(c == HC), stop=False,
                    )

        # diag(mask_j/len) stationaries for the PE mean: 8 per half, built
        # with cheap packed DVE tensor_scalar ops from diag(1/len) x m8[j].
        m8 = cst32_t[:, NE + 2:NE + 2 + 16]
        dmask = [pool.tile([128, SPAN * 128], F16, name=f"dmask{h}")
                 for h in range(2)]
        for h in range(2):
            diag = dcst_t[:, h * 128:(h + 1) * 128]
            for j in range(SPAN):
                nc.vector.tensor_scalar(
                    dmask[h][:, j * 128:(j + 1) * 128], diag,
                    m8[:, 8 * h + j:8 * h + j + 1], None,
                    op0=mybir.AluOpType.mult)

        featT_pairs = featT[:].rearrange("p (c x) -> p c x", x=256)
        hiddenT = pool.tile([128, 2 * 256], F16)
        identity = dcst_t[:, 256:384]
        out3 = pool.tile([128, 2 * NB_LABELS], F32)  # [p, h*3 + n]
        outv = out.rearrange("(h p) n -> p h n", p=128)

        ctx_mms(0)
        ctx_mms(1)

        # masked mean on the PE, both halves back to back so h1's matmuls
        # never queue behind h0's downstream chain
        ps_h = []
        for h in range(2):
            g = g_h[h]
            psa = pst.tile([128, 512], F32, tag=f"psa{h}", bufs=1)
            psb = pst.tile([128, 256], F32, tag="psb", bufs=1, name=f"psb{h}")
            for j in range(SPAN):
                dm = dmask[h][:, j * 128:(j + 1) * 128]
                nc.tensor.matmul(psa[:], lhsT=dm, rhs=g[:, j * H:j * H + 512],
                                 start=(j == 0), stop=(j == SPAN - 1))
                nc.tensor.matmul(psb[:], lhsT=dm,
                                 rhs=g[:, j * H + 512:(j + 1) * H],
                                 start=(j == 0), stop=(j == SPAN - 1))
            ps_h.append((psa, psb))

        for h in range(2):
            psa, psb = ps_h[h]
            ct = gpool.tile([128, H], F16, tag=f"ct{h}", bufs=1)
            nc.vector.tensor_copy(ct[:, 512:768], psb[:])
            nc.vector.tensor_copy(ct[:, 0:512], psa[:])

            # center transposes (PE transpose mode, identity permutation):
            # all 6 chunks land in one PSUM bank, drained by a single strided
            # copy into featT
            tpc = pst.tile([128, HC * 128], F16, tag="tpc", bufs=1)
            for c in range(HC):
                nc.tensor.transpose(tpc[:, c * 128:(c + 1) * 128],
                                    ct[:, c * 128:(c + 1) * 128], identity)
            nc.scalar.copy(
                featT_pairs[:, 0:HC, h * 128:(h + 1) * 128],
                tpc[:].rearrange("p (c x) -> p c x", c=HC))

            # center chunks close the base-linear accumulation; bias+relu
            for c in range(HC):
                for mt in range(2):
                    nc.tensor.matmul(
                        accs_h[h][mt][:],
                        lhsT=wbT_t[:, c * INNER + mt * 128: c * INNER + (mt + 1) * 128],
                        rhs=featT[:, c * 256 + h * 128: c * 256 + h * 128 + 128],
                        start=False, stop=(c == HC - 1),
                    )
            for mt in range(2):
                nc.scalar.activation(
                    hiddenT[:, mt * 256 + h * 128: mt * 256 + h * 128 + 128],
                    accs_h[h][mt][:],
                    mybir.ActivationFunctionType.Relu,
                    bias=bb_t[:, mt:mt + 1], scale=1.0)

            # expert heads + per-sample selection, inline per half
            b0 = h * 128
            mask36 = spool.tile([128, NE], F32, tag=f"mask36{h}", bufs=1)
            nc.vector.tensor_scalar(mask36[:], io36f, catf[:, h:h + 1], None,
                                    op0=mybir.AluOpType.is_equal)
            ps36 = ps_h[h][0][:, 0:NE]
            nc.tensor.matmul(ps36, lhsT=hiddenT[:, b0:b0 + 128],
                             rhs=wexpA, start=True, stop=False)
            nc.tensor.matmul(ps36, lhsT=hiddenT[:, 256 + b0:256 + b0 + 128],
                             rhs=wexpB, start=False, stop=False)
            nc.tensor.matmul(ps36, lhsT=ones1[0:1, b0:b0 + 128],
                             rhs=wexpC, start=False, stop=True)

            prod = spool.tile([128, NE], F32, tag=f"prod{h}", bufs=1)
            nc.vector.tensor_tensor(out=prod[:], in0=ps36, in1=mask36[:],
                                    op=mybir.AluOpType.mult)
            nc.vector.tensor_reduce(
                out=out3[:, h * NB_LABELS:(h + 1) * NB_LABELS],
                in_=prod[:].rearrange("p (e n) -> p n e", n=NB_LABELS),
                axis=mybir.AxisListType.X, op=mybir.AluOpType.add)
            nc.sync.dma_start(
                outv[:, h:h + 1, :],
                out3[:].rearrange("p (g n) -> p g n", n=NB_LABELS)[:, h:h + 1, :])

    nc.compile()
    return nc


_NC = None


def _get_nc():
    global _NC
    if _NC is None:
        _NC = _build()
    return _NC


def _prep_inputs(embeddings, position_indexes, categories, W_base, b_base,
                 W_experts, b_experts):
    emb32 = np.asarray(embeddings)
    emb16 = emb32.astype(np.float16).reshape(NCORES, 2, HROWS, H)

    pos = np.asarray(position_indexes).astype(np.int64).reshape(NCORES, BC, 2)
    cat = np.asarray(categories).astype(np.int64).reshape(NCORES, BC)

    # gather start rows [128, 2]: col h = span-start row of sample h*128+p
    # within that half's emb tensor (8 contiguous rows always fit: spans lie
    # inside [0, S) and len <= 8).
    starts = pos[:, :, 0].reshape(NCORES, 2, 128)
    lens = (pos[:, :, 1] - pos[:, :, 0]).reshape(NCORES, 2, 128)
    base = np.arange(128, dtype=np.int64) * S
    rowA = base[None, None, :] + starts                       # [NC, 2, 128]
    gidx = rowA.transpose(0, 2, 1).reshape(NCORES, 128, 2).astype(np.int32)

    rcp = 1.0 / lens.astype(np.float32)  # [NC, 2, 128]
    eye = np.eye(128, dtype=np.float32)
    diags = (eye[None, None] * rcp[:, :, :, None]).transpose(0, 2, 1, 3).reshape(
        NCORES, 128, 256)
    ident = np.broadcast_to(eye[None], (NCORES, 128, 128))

    # base linear: wbT_host[p, c*INNER+m] = W_base[m, c*128+p]
    wb = np.asarray(W_base, dtype=np.float32)  # [INNER, 3H]
    wbT = np.ascontiguousarray(
        wb.T.reshape(KC, 128, INNER).transpose(1, 0, 2).reshape(128, KC * INNER)
    ).astype(np.float16).reshape(F3H, INNER)

    bbias = np.ascontiguousarray(np.asarray(b_base, dtype=np.float32))

    we = np.asarray(W_experts, dtype=np.float32)  # [12, 3, INNER]
    be = np.asarray(b_experts, dtype=np.float32)  # [12, 3]
    wexp = we.transpose(2, 0, 1).reshape(INNER, NE)  # row m -> experts
    c16 = np.concatenate(
        [diags, ident,
         np.broadcast_to(wexp[None, 0:128], (NCORES, 128, NE)),
         np.broadcast_to(wexp[None, 128:256], (NCORES, 128, NE))],
        axis=2).astype(np.float16)
    c1 = np.concatenate(
        [np.ones((1, 256), dtype=np.float32), be.reshape(1, NE)],
        axis=1).astype(np.float16)

    # static context rows, pre-transposed into featT layout:
    # ctxT[p, (which*6+cc)*256 + h*128 + sl] = emb[s(h,sl), CTX_IDX[which], cc*128+p]
    emb3 = emb16.reshape(NCORES, BC, S, H)
    blocks = []
    for which in range(NB_CTX):
        blk = emb3[:, :, CTX_IDX[which], :]                     # [NC, 256, 768]
        arr = blk.reshape(NCORES, 2, 128, HC, 128).transpose(0, 4, 3, 1, 2)
        blocks.append(arr.reshape(NCORES, 128, HC * 256))
    ctxT = np.ascontiguousarray(np.concatenate(blocks, axis=2))  # [NC, 128, 12*256]

    # io36 + categories-as-float + span masks + b_base (f32)
    cst32 = np.zeros((NCORES, 128, NE + 2 + 16 + 2), dtype=np.float32)
    cst32[:, :, :NE] = np.repeat(np.arange(NB_EXPERTS, dtype=np.float32),
                                 NB_LABELS)[None, None, :]
    cst32[:, :, NE:NE + 2] = cat.reshape(NCORES, 2, 128).transpose(0, 2, 1)
    j = np.arange(SPAN, dtype=np.int64)
    m8 = (j[None, None, None, :] < lens[..., None]).astype(np.float32)
    cst32[:, :, NE + 2:NE + 18] = m8.transpose(0, 2, 1, 3).reshape(NCORES, 128, 16)
    cst32[:, :, NE + 18:] = bbias.reshape(2, 128).T[None]

    return [
        {"emb0": np.ascontiguousarray(emb16[i, 0]),
         "emb1": np.ascontiguousarray(emb16[i, 1]),
         "gidx": np.ascontiguousarray(gidx[i]),
         "wbT": wbT, "ctxT": ctxT[i],
         "c16": np.ascontiguousarray(c16[i]),
         "c32": np.ascontiguousarray(cst32[i]),
         "c1": np.ascontiguousarray(c1)}
        for i in range(NCORES)
    ]


def _run(in_maps, **kw):
    nc = _get_nc()
    return run_bass_kernel_spmd(nc, in_maps, core_ids=list(range(NCORES)), **kw)


def kernel(embeddings, position_indexes, categories, W_base, b_base, W_experts,
           b_experts):
    in_maps = _prep_inputs(embeddings, position_indexes, categories, W_base,
                           b_base, W_experts, b_experts)
    res = _run(in_maps)
    return np.concatenate([r["out"] for r in res.results], axis=0)

